# revision 27
# baseline (speedup 1.0000x reference)
"""Isolated single-head attention on 8 Trainium2 NeuronCores.

Problem: inp_emb (4, 4096, 1024) f32; Wq/Wk/Wv (1024, 1024) f32.
  Q = x @ Wq.T; K = x @ Wk.T; V = x @ Wv.T
  out = softmax(Q K^T / 32) @ V          (per batch)

Sharding: core c -> batch b = c//2, q-row half h = c%2 (2048 rows).
Keys are reordered per-core as [own-half rows, buddy-half rows] (softmax
is permutation-invariant over keys) so every address is static SPMD.

Algebraic restructure (removes the K projection and the duplicated
full-batch V projection entirely):
  A  = Wq^T @ Wk  (host, f32 -> bf16; shared across cores)
  S  = Q K^T = x_q A x^T          -> QA = x_q @ A   (own rows only)
  O  = P V  = (P x) Wv^T          -> PXT = x^T P^T, O = PX @ Wv^T
Per-core matmul work: QA 4.3 + S 17.2 + PX 17.2 + O-proj 4.3 GFLOP
(vs 55.8 GFLOP for the direct form with duplicated K/V).

All matmuls bf16 with f32 PSUM accumulation. Layouts per core:
  xt  [1024 d, 4096 k] bf16  resident SBUF (scores lhsT + QA rhs)
  xr  [4096 k, 1024 d] bf16  resident SBUF (PXT lhsT)
  at  [1024 d, 1024 j] bf16  A            (QA lhsT; slot reused by EXP)
  wvt [1024 d, 1024 e] bf16  Wv^T         (O-proj rhs, resident)
Kernel phases (per core, fully unrolled):
  QA pass: QAT[j, q] -> DRAM scratch (SBUF is full)
  Attention per 512-row q-block:
    ST[k, q] = xt-tile.T @ QAT-block   (PSUM f32, 8 accum MMs)
    EXP[k, q] = exp(ST/32) -> SBUF bf16 (no max subtraction: |s|<~6)
    sum[1, q] += ones.T @ EXP          (matmul, PSUM accumulate)
    PXT[d, q] = sum_k xr-tile.T @ EXP  (PSUM f32 -> SBUF bf16)
    per q-slice of 128: transpose sums via Kc=1 matmul, reciprocal,
    O[q, e] = sum_d pxt-tile.T @ wvt, scaled by 1/sum on ScalarE,
    stored bf16 (host casts back to f32).
"""

import numpy as np
import ml_dtypes

D = 1024
S = 4096          # keys per batch
SQ = 2048         # q rows per core
QB = 512          # q-block
NQB = SQ // QB    # 4
NKT = S // 128    # 32 k tiles
ND = D // 128     # 8 chunks of d/j/e
SCALE = 1.0 / 32.0

_CACHE = {}
TRACE = False
LAST_RESULT = None


def _build():
    import concourse.bass as bass
    import concourse.bacc as bacc
    import concourse.mybir as mybir
    import concourse.tile as tile

    f32 = mybir.dt.float32
    bf16 = mybir.dt.bfloat16
    EXPF = mybir.ActivationFunctionType.Exp

    nc = bacc.Bacc(None)
    xt_d = nc.dram_tensor("xt", [D, S], bf16, kind="ExternalInput")
    xr_d = nc.dram_tensor("xr", [S, D], bf16, kind="ExternalInput")
    at_d = nc.dram_tensor("at", [D, D], bf16, kind="ExternalInput")
    wvt_d = nc.dram_tensor("wvt", [D, D], bf16, kind="ExternalInput")
    out_d = nc.dram_tensor("out", [SQ, D], bf16, kind="ExternalOutput")

    with tile.TileContext(nc) as tc:
        with (
            tc.tile_pool(name="xtp", bufs=1) as xtp,
            tc.tile_pool(name="xrp", bufs=1) as xrp,
            tc.tile_pool(name="wvp", bufs=1) as wvp,
            tc.tile_pool(name="big", bufs=2) as bigp,
            tc.tile_pool(name="qtp", bufs=2) as qtp,
            tc.tile_pool(name="pxp", bufs=1) as pxp,
            tc.tile_pool(name="stg", bufs=3) as stgp,
            tc.tile_pool(name="sml", bufs=1) as smlp,
            tc.tile_pool(name="acc", bufs=1) as accp,
            tc.tile_pool(name="cst", bufs=1) as cstp,
            tc.tile_pool(name="psQ", bufs=2, space="PSUM") as psQ,
            tc.tile_pool(name="psB", bufs=2, space="PSUM") as psB,
            tc.tile_pool(name="psX", bufs=2, space="PSUM") as psX,
            tc.tile_pool(name="psS", bufs=1, space="PSUM") as psS,
            tc.tile_pool(name="psR", bufs=1, space="PSUM") as psR,
            tc.tile_pool(name="drm", bufs=1, space="DRAM") as drmp,
        ):
            xt_sb = xtp.tile([128, ND, S], bf16)      # xT[d, k]: 64KB/part
            xr_sb = xrp.tile([128, NKT, D], bf16)     # x[k, d]:  64KB/part
            wvt_sb = wvp.tile([128, ND, D], bf16)     # WvT[d, e]: 16KB/part
            qat_dram = drmp.tile([D, SQ], bf16)

            ones_sb = cstp.tile([128, 1], f32)
            one_f32 = cstp.tile([1, 1], f32)
            nc.vector.memset(ones_sb[:], 1.0)
            nc.vector.memset(one_f32[:], 1.0)

            # A first (QA needs it), then xt own-half pieces. The SWDGE ring
            # is FIFO: only tensors QA consumes go here so the QA staging
            # stores aren't stuck behind bulk loads. xr/wvt go on the HWDGE
            # ring (emitted after the QA pass; not needed until attention).
            a_sb = bigp.tile([128, ND, D], bf16, tag="big")
            at_re = at_d.rearrange("(c p) j -> p c j", p=128)
            xt_re = xt_d.rearrange("(j p) k -> p j k", p=128)
            # Ring order matches first-use order: the first QA group needs
            # xt cols 0:512 plus only the low j columns of A (j-quarters).
            nc.gpsimd.dma_start(out=xt_sb[:, :, 0:512], in_=xt_re[:, :, 0:512])
            for jq in range(4):
                nc.gpsimd.dma_start(
                    out=a_sb[:, :, jq * 256:(jq + 1) * 256],
                    in_=at_re[:, :, jq * 256:(jq + 1) * 256],
                )
            for kc in range(1, S // 512):
                nc.gpsimd.dma_start(
                    out=xt_sb[:, :, kc * 512:(kc + 1) * 512],
                    in_=xt_re[:, :, kc * 512:(kc + 1) * 512],
                )

            # ---------------- QA pass: QAT[j, q] ----------------
            # qc 0 keeps its result in SBUF (used by the first attention
            # q-block with no DRAM roundtrip); qc 1..3 stage via DRAM.
            qt0 = qtp.tile([128, ND, QB], bf16, tag="qt", name="qt0")
            for qc in range(SQ // 512):
                for j in range(ND):
                    # Alternate the two (otherwise idle) PSUM pools for a
                    # 4-deep accumulator pipeline during the QA pass.
                    pool, ptag = (psQ, "psQ") if j % 2 == 0 else (psX, "psX")
                    ps = pool.tile([128, 512], f32, tag=ptag)
                    for dc in range(ND):
                        qa_mm = nc.tensor.matmul(
                            ps[:],
                            a_sb[:, dc, j * 128:(j + 1) * 128],
                            xt_sb[:, dc, qc * 512:(qc + 1) * 512],
                            start=(dc == 0), stop=(dc == ND - 1),
                        )
                    if qc == 0:
                        nc.vector.tensor_copy(qt0[:, j, :], ps[:])
                    else:
                        st = stgp.tile([128, 512], bf16, tag="stg")
                        nc.vector.tensor_copy(st[:], ps[:])
                        nc.gpsimd.dma_start(
                            out=qat_dram[j * 128:(j + 1) * 128,
                                         qc * 512:(qc + 1) * 512],
                            in_=st[:],
                        )

            # Bulk loads for the attention phase: HWDGE ring, gated behind
            # the whole QA pass so they don't fight the QA loads/stores for
            # HBM bandwidth (xr is first needed ~55us after QA ends).
            xr_dma = nc.sync.dma_start(
                out=xr_sb[:], in_=xr_d.rearrange("(t p) d -> p t d", p=128)
            )
            bass._add_dep_helper(
                xr_dma.ins, qa_mm.ins, reason="delay xr load past the QA pass"
            )
            nc.sync.dma_start(
                out=wvt_sb[:], in_=wvt_d.rearrange("(c p) e -> p c e", p=128)
            )

            # ---------------- Attention ----------------
            for qb in range(NQB):
                q0 = qb * QB
                if qb == 0:
                    qt = qt0
                else:
                    qt = qtp.tile([128, ND, QB], bf16, tag="qt")
                    nc.gpsimd.dma_start(
                        out=qt[:],
                        in_=qat_dram[:].rearrange("(j p) q -> p j q", p=128)[
                            :, :, q0:q0 + QB
                        ],
                    )
                # EXP for this q-block lives in two 16-kt half tiles that
                # recycle the big pool's slots (a_sb is dead after QA).
                exp_h = [
                    bigp.tile([128, NKT // 2, QB], bf16, tag="big", name=f"exp{qb}_{i}")
                    for i in range(2)
                ]
                acc = accp.tile([128, QB], f32, tag="acc")
                for kt in range(NKT):
                    eh = exp_h[kt // 16]
                    ps_st = psB.tile([128, QB], f32, tag="psB")
                    for j in range(ND):
                        nc.tensor.matmul(
                            ps_st[:],
                            xt_sb[:, j, kt * 128:(kt + 1) * 128],
                            qt[:, j, :],
                            start=(j == 0), stop=(j == ND - 1),
                        )
                    nc.scalar.activation(eh[:, kt % 16, :], ps_st[:], EXPF, scale=SCALE)
                    # Partial key-tile sum on the (otherwise idle) DVE in f32
                    # — same precision as PSUM accumulation. The partition
                    # reduction then needs just ONE matmul instead of 32.
                    if kt == 0:
                        nc.vector.tensor_copy(acc[:], eh[:, 0, :])
                    else:
                        nc.vector.tensor_add(acc[:], acc[:], eh[:, kt % 16, :])

                # PXT[d, q] = sum_k x[k, d] * EXP[k, q]
                pxt_sb = pxp.tile([128, ND, QB], bf16, tag="pxt")
                for dt in range(ND):
                    ps_px = psX.tile([128, QB], f32, tag="psX")
                    for kt in range(NKT):
                        nc.tensor.matmul(
                            ps_px[:],
                            xr_sb[:, kt, dt * 128:(dt + 1) * 128],
                            exp_h[kt // 16][:, kt % 16, :],
                            start=(kt == 0), stop=(kt == NKT - 1),
                        )
                    nc.vector.tensor_copy(pxt_sb[:, dt, :], ps_px[:])

                for qs in range(QB // 128):
                    # Per-q sums, already transposed to [q, 1]: acc slice is
                    # the stationary operand, ones column the moving one.
                    ps_r = psR.tile([128, 1], f32, tag="psR")
                    nc.tensor.matmul(
                        ps_r[:], acc[:, qs * 128:(qs + 1) * 128], ones_sb[:],
                        start=True, stop=True,
                    )
                    sumt = smlp.tile([128, 1], f32, tag="sumt")
                    nc.vector.tensor_copy(sumt[:], ps_r[:])
                    rcp = smlp.tile([128, 1], f32, tag="rcp")
                    nc.vector.reciprocal(rcp[:], sumt[:])
                    for ec in range(2):
                        ps_o = psQ.tile([128, 512], f32, tag="psQ")
                        for dt in range(ND):
                            nc.tensor.matmul(
                                ps_o[:],
                                pxt_sb[:, dt, qs * 128:(qs + 1) * 128],
                                wvt_sb[:, dt, ec * 512:(ec + 1) * 512],
                                start=(dt == 0), stop=(dt == ND - 1),
                            )
                        o_sb = stgp.tile([128, 512], bf16, tag="stg")
                        nc.scalar.mul(o_sb[:], ps_o[:], rcp[:])
                        nc.gpsimd.dma_start(
                            out=out_d[q0 + qs * 128:q0 + qs * 128 + 128,
                                      ec * 512:(ec + 1) * 512],
                            in_=o_sb[:],
                        )
    nc.compile()
    return nc


def kernel(inp_emb, Wq, Wk, Wv):
    global LAST_RESULT
    from concourse.bass_utils import run_bass_kernel_spmd

    bf = ml_dtypes.bfloat16
    x = np.asarray(inp_emb, dtype=np.float32)
    a = np.ascontiguousarray(
        np.asarray(Wq, np.float32).T @ np.asarray(Wk, np.float32)
    ).astype(bf)
    wvt = np.ascontiguousarray(np.asarray(Wv, np.float32).T).astype(bf)

    in_maps = []
    for c in range(8):
        b, h = divmod(c, 2)
        x_re = np.concatenate(
            [x[b, h * SQ:(h + 1) * SQ], x[b, (1 - h) * SQ:(2 - h) * SQ]], axis=0
        )
        xr = np.ascontiguousarray(x_re).astype(bf)          # (4096, 1024)
        xt = np.ascontiguousarray(x_re.T).astype(bf)        # (1024, 4096)
        in_maps.append({"xt": xt, "xr": xr, "at": a, "wvt": wvt})

    if "nc" not in _CACHE:
        _CACHE["nc"] = _build()
    nc = _CACHE["nc"]

    res = run_bass_kernel_spmd(nc, in_maps, list(range(8)), trace=TRACE)
    LAST_RESULT = res

    out = np.empty((4, S, D), dtype=np.float32)
    for c in range(8):
        b, h = divmod(c, 2)
        out[b, h * SQ:(h + 1) * SQ] = res.results[c]["out"].astype(np.float32)
    return out


# revision 28
# speedup vs baseline: 1.1743x; 1.1743x over previous
"""Isolated single-head attention on 8 Trainium2 NeuronCores.

Problem: inp_emb (4, 4096, 1024) f32; Wq/Wk/Wv (1024, 1024) f32.
  Q = x @ Wq.T; K = x @ Wk.T; V = x @ Wv.T
  out = softmax(Q K^T / 32) @ V          (per batch)

Sharding: core c -> batch b = c//2, q-row half h = c%2 (2048 rows).
Keys are reordered per-core as [own-half rows, buddy-half rows] (softmax
is permutation-invariant over keys) so every address is static SPMD.

Algebraic restructure (removes the K projection and the duplicated
full-batch V projection entirely):
  A  = Wq^T @ Wk  (host, f32 -> bf16; shared across cores)
  S  = Q K^T = x_q A x^T          -> QA = x_q @ A   (own rows only)
  O  = P V  = (P x) Wv^T          -> PXT = x^T P^T, O = PX @ Wv^T
Per-core matmul work: QA 4.3 + S 17.2 + PX 17.2 + O-proj 4.3 GFLOP
(vs 55.8 GFLOP for the direct form with duplicated K/V).

All matmuls bf16 with f32 PSUM accumulation. Layouts per core:
  xt  [1024 d, 4096 k] bf16  resident SBUF (scores lhsT + QA rhs)
  xr  [4096 k, 1024 d] bf16  resident SBUF (PXT lhsT)
  at  [1024 d, 1024 j] bf16  A            (QA lhsT; slot reused by EXP)
  wvt [1024 d, 1024 e] bf16  Wv^T         (O-proj rhs, resident)
Kernel phases (per core, fully unrolled):
  QA pass: QAT[j, q] -> DRAM scratch (SBUF is full)
  Attention per 512-row q-block:
    ST[k, q] = xt-tile.T @ QAT-block   (PSUM f32, 8 accum MMs)
    EXP[k, q] = exp(ST/32) -> SBUF bf16 (no max subtraction: |s|<~6)
    sum[1, q] += ones.T @ EXP          (matmul, PSUM accumulate)
    PXT[d, q] = sum_k xr-tile.T @ EXP  (PSUM f32 -> SBUF bf16)
    per q-slice of 128: transpose sums via Kc=1 matmul, reciprocal,
    O[q, e] = sum_d pxt-tile.T @ wvt, scaled by 1/sum on ScalarE,
    stored bf16 (host casts back to f32).
"""

import numpy as np
import ml_dtypes

D = 1024
S = 4096          # keys per batch
SQ = 2048         # q rows per core
QB = 512          # q-block
NQB = SQ // QB    # 4
NKT = S // 128    # 32 k tiles
ND = D // 128     # 8 chunks of d/j/e
SCALE = 1.0 / 32.0

_CACHE = {}
TRACE = False
LAST_RESULT = None


def _build():
    import concourse.bass as bass
    import concourse.bacc as bacc
    import concourse.mybir as mybir
    import concourse.tile as tile

    f32 = mybir.dt.float32
    bf16 = mybir.dt.bfloat16
    EXPF = mybir.ActivationFunctionType.Exp

    nc = bacc.Bacc(None)
    xt_d = nc.dram_tensor("xt", [D, S], bf16, kind="ExternalInput")
    xr_d = nc.dram_tensor("xr", [S, D], bf16, kind="ExternalInput")
    at_d = nc.dram_tensor("at", [D, D], bf16, kind="ExternalInput")
    wvt_d = nc.dram_tensor("wvt", [D, D], bf16, kind="ExternalInput")
    out_d = nc.dram_tensor("out", [SQ, D], bf16, kind="ExternalOutput")

    with tile.TileContext(nc) as tc:
        with (
            tc.tile_pool(name="xtp", bufs=1) as xtp,
            tc.tile_pool(name="xrp", bufs=1) as xrp,
            tc.tile_pool(name="wvp", bufs=1) as wvp,
            tc.tile_pool(name="big", bufs=2) as bigp,
            tc.tile_pool(name="qtp", bufs=2) as qtp,
            tc.tile_pool(name="pxp", bufs=1) as pxp,
            tc.tile_pool(name="stg", bufs=3) as stgp,
            tc.tile_pool(name="sml", bufs=1) as smlp,
            tc.tile_pool(name="acc", bufs=1) as accp,
            tc.tile_pool(name="cst", bufs=1) as cstp,
            tc.tile_pool(name="psQ", bufs=2, space="PSUM") as psQ,
            tc.tile_pool(name="psB", bufs=2, space="PSUM") as psB,
            tc.tile_pool(name="psX", bufs=2, space="PSUM") as psX,
            tc.tile_pool(name="psS", bufs=1, space="PSUM") as psS,
            tc.tile_pool(name="psR", bufs=1, space="PSUM") as psR,
            tc.tile_pool(name="drm", bufs=1, space="DRAM") as drmp,
        ):
            xt_sb = xtp.tile([128, ND, S], bf16)      # xT[d, k]: 64KB/part
            xr_sb = xrp.tile([128, NKT, D], bf16)     # x[k, d]:  64KB/part
            wvt_sb = wvp.tile([128, ND, D], bf16)     # WvT[d, e]: 16KB/part
            qat_dram = drmp.tile([D, SQ], bf16)

            ones_sb = cstp.tile([128, 1], f32)
            one_f32 = cstp.tile([1, 1], f32)
            nc.vector.memset(ones_sb[:], 1.0)
            nc.vector.memset(one_f32[:], 1.0)

            # A first (QA needs it), then xt own-half pieces. The SWDGE ring
            # is FIFO: only tensors QA consumes go here so the QA staging
            # stores aren't stuck behind bulk loads. xr/wvt go on the HWDGE
            # ring (emitted after the QA pass; not needed until attention).
            a_sb = bigp.tile([128, ND, D], bf16, tag="big")
            at_re = at_d.rearrange("(c p) j -> p c j", p=128)
            xt_re = xt_d.rearrange("(j p) k -> p j k", p=128)
            # Ring order matches first-use order: the first QA group needs
            # xt cols 0:512 plus only the low j columns of A (j-quarters).
            nc.gpsimd.dma_start(out=xt_sb[:, :, 0:512], in_=xt_re[:, :, 0:512])
            for jq in range(4):
                nc.gpsimd.dma_start(
                    out=a_sb[:, :, jq * 256:(jq + 1) * 256],
                    in_=at_re[:, :, jq * 256:(jq + 1) * 256],
                )
            for kc in range(1, S // 512):
                nc.gpsimd.dma_start(
                    out=xt_sb[:, :, kc * 512:(kc + 1) * 512],
                    in_=xt_re[:, :, kc * 512:(kc + 1) * 512],
                )

            # ---------------- QA pass: QAT[j, q] ----------------
            # qc 0 keeps its result in SBUF (used by the first attention
            # q-block with no DRAM roundtrip); qc 1..3 stage via DRAM.
            qt0 = qtp.tile([128, ND, QB], bf16, tag="qt", name="qt0")
            for qc in range(SQ // 512):
                for j in range(ND):
                    # Alternate the two (otherwise idle) PSUM pools for a
                    # 4-deep accumulator pipeline during the QA pass.
                    pool, ptag = (psQ, "psQ") if j % 2 == 0 else (psX, "psX")
                    ps = pool.tile([128, 512], f32, tag=ptag)
                    for dc in range(ND):
                        qa_mm = nc.tensor.matmul(
                            ps[:],
                            a_sb[:, dc, j * 128:(j + 1) * 128],
                            xt_sb[:, dc, qc * 512:(qc + 1) * 512],
                            start=(dc == 0), stop=(dc == ND - 1),
                        )
                    if qc == 0:
                        nc.vector.tensor_copy(qt0[:, j, :], ps[:])
                    else:
                        st = stgp.tile([128, 512], bf16, tag="stg")
                        nc.vector.tensor_copy(st[:], ps[:])
                        nc.gpsimd.dma_start(
                            out=qat_dram[j * 128:(j + 1) * 128,
                                         qc * 512:(qc + 1) * 512],
                            in_=st[:],
                        )

            # Bulk loads for the attention phase: HWDGE ring, gated behind
            # the whole QA pass so they don't fight the QA loads/stores for
            # HBM bandwidth (xr is first needed ~55us after QA ends).
            xr_dma = nc.sync.dma_start(
                out=xr_sb[:], in_=xr_d.rearrange("(t p) d -> p t d", p=128)
            )
            bass._add_dep_helper(
                xr_dma.ins, qa_mm.ins, reason="delay xr load past the QA pass"
            )
            nc.sync.dma_start(
                out=wvt_sb[:], in_=wvt_d.rearrange("(c p) e -> p c e", p=128)
            )

            # ---------------- Attention ----------------
            for qb in range(NQB):
                q0 = qb * QB
                if qb == 0:
                    qt = qt0
                else:
                    qt = qtp.tile([128, ND, QB], bf16, tag="qt")
                    nc.gpsimd.dma_start(
                        out=qt[:],
                        in_=qat_dram[:].rearrange("(j p) q -> p j q", p=128)[
                            :, :, q0:q0 + QB
                        ],
                    )
                # EXP for this q-block lives in two 16-kt half tiles that
                # recycle the big pool's slots (a_sb is dead after QA).
                exp_h = [
                    bigp.tile([128, NKT // 2, QB], bf16, tag="big", name=f"exp{qb}_{i}")
                    for i in range(2)
                ]
                acc = accp.tile([128, QB], f32, tag="acc")
                for kt in range(NKT):
                    eh = exp_h[kt // 16]
                    ps_st = psB.tile([128, QB], f32, tag="psB")
                    for j in range(ND):
                        nc.tensor.matmul(
                            ps_st[:],
                            xt_sb[:, j, kt * 128:(kt + 1) * 128],
                            qt[:, j, :],
                            start=(j == 0), stop=(j == ND - 1),
                        )
                    nc.scalar.activation(eh[:, kt % 16, :], ps_st[:], EXPF, scale=SCALE)
                    # Partial key-tile sum on the (otherwise idle) DVE in f32
                    # — same precision as PSUM accumulation. The partition
                    # reduction then needs just ONE matmul instead of 32.
                    if kt == 0:
                        nc.vector.tensor_copy(acc[:], eh[:, 0, :])
                    else:
                        nc.vector.tensor_add(acc[:], acc[:], eh[:, kt % 16, :])

                # PXT[d, q] = sum_k x[k, d] * EXP[k, q]
                pxt_sb = pxp.tile([128, ND, QB], bf16, tag="pxt")
                for dt in range(ND):
                    ps_px = psX.tile([128, QB], f32, tag="psX")
                    for kt in range(NKT):
                        nc.tensor.matmul(
                            ps_px[:],
                            xr_sb[:, kt, dt * 128:(dt + 1) * 128],
                            exp_h[kt // 16][:, kt % 16, :],
                            start=(kt == 0), stop=(kt == NKT - 1),
                        )
                    nc.vector.tensor_copy(pxt_sb[:, dt, :], ps_px[:])

                ps_sum = psS.tile([1, QB], f32, tag="psS")
                nc.tensor.matmul(
                    ps_sum[:], ones_sb[:], acc[:], start=True, stop=True
                )
                sum_sb = smlp.tile([1, QB], f32, tag="sum")
                nc.vector.tensor_copy(sum_sb[:], ps_sum[:])
                for qs in range(QB // 128):
                    ps_r = psR.tile([128, 1], f32, tag="psR")
                    nc.tensor.matmul(
                        ps_r[:], sum_sb[0:1, qs * 128:(qs + 1) * 128], one_f32[:],
                        start=True, stop=True,
                    )
                    sumt = smlp.tile([128, 1], f32, tag="sumt")
                    nc.vector.tensor_copy(sumt[:], ps_r[:])
                    rcp = smlp.tile([128, 1], f32, tag="rcp")
                    nc.vector.reciprocal(rcp[:], sumt[:])
                    for ec in range(2):
                        ps_o = psQ.tile([128, 512], f32, tag="psQ")
                        for dt in range(ND):
                            nc.tensor.matmul(
                                ps_o[:],
                                pxt_sb[:, dt, qs * 128:(qs + 1) * 128],
                                wvt_sb[:, dt, ec * 512:(ec + 1) * 512],
                                start=(dt == 0), stop=(dt == ND - 1),
                            )
                        o_sb = stgp.tile([128, 512], bf16, tag="stg")
                        nc.scalar.mul(o_sb[:], ps_o[:], rcp[:])
                        nc.gpsimd.dma_start(
                            out=out_d[q0 + qs * 128:q0 + qs * 128 + 128,
                                      ec * 512:(ec + 1) * 512],
                            in_=o_sb[:],
                        )
    nc.compile()
    return nc


def kernel(inp_emb, Wq, Wk, Wv):
    global LAST_RESULT
    from concourse.bass_utils import run_bass_kernel_spmd

    bf = ml_dtypes.bfloat16
    x = np.asarray(inp_emb, dtype=np.float32)
    a = np.ascontiguousarray(
        np.asarray(Wq, np.float32).T @ np.asarray(Wk, np.float32)
    ).astype(bf)
    wvt = np.ascontiguousarray(np.asarray(Wv, np.float32).T).astype(bf)

    in_maps = []
    for c in range(8):
        b, h = divmod(c, 2)
        x_re = np.concatenate(
            [x[b, h * SQ:(h + 1) * SQ], x[b, (1 - h) * SQ:(2 - h) * SQ]], axis=0
        )
        xr = np.ascontiguousarray(x_re).astype(bf)          # (4096, 1024)
        xt = np.ascontiguousarray(x_re.T).astype(bf)        # (1024, 4096)
        in_maps.append({"xt": xt, "xr": xr, "at": a, "wvt": wvt})

    if "nc" not in _CACHE:
        _CACHE["nc"] = _build()
    nc = _CACHE["nc"]

    res = run_bass_kernel_spmd(nc, in_maps, list(range(8)), trace=TRACE)
    LAST_RESULT = res

    out = np.empty((4, S, D), dtype=np.float32)
    for c in range(8):
        b, h = divmod(c, 2)
        out[b, h * SQ:(h + 1) * SQ] = res.results[c]["out"].astype(np.float32)
    return out
